# revision 3
# baseline (speedup 1.0000x reference)
"""Nearest-exemplar kNN scores on 8 Trainium2 NeuronCores.

Computes class_scores[b, m, k] = max_n( -||test[b,m] - train[b,k,n]||^2 )
data-parallel over the episode dim b (8 episodes per core).

Device kernel (per core, 8 episodes):
  psum[128m, 512kn] = sum_d testT[d, m].T @ (2*trainT[d, kn])   (4 matmuls)
                    + ones[1,128m].T @ (-train_sq[1, 512kn])    (K=1 matmul)
  out[m, k] = reduce_max over the 16 shots (innermost free axis)
The -test_sq[b, m] term is constant across (k, n), so it commutes with the
max and is applied on the host after gathering.
"""

import numpy as np

import concourse.bass as bass
import concourse.tile as tile
from concourse import bacc, mybir
from concourse.bass_utils import run_bass_kernel_spmd

B, K, N, M, D = 64, 64, 16, 512, 512
KN = K * N            # 1024 support vectors per episode
NCORES = 8
BPC = B // NCORES     # 8 episodes per core
P = 128
DT = D // P           # 4 contraction tiles
MT = M // P           # 4 query tiles
NKC = KN // 512       # 2 support chunks of 512

_compiled = None


def _build():
    nc = bacc.Bacc("TRN2", target_bir_lowering=False, debug=False, num_devices=1)
    f32 = mybir.dt.float32
    train2t = nc.dram_tensor("train2t", [BPC, D, KN], f32, kind="ExternalInput").ap()
    testt = nc.dram_tensor("testt", [BPC, D, M], f32, kind="ExternalInput").ap()
    nsq = nc.dram_tensor("nsq", [1, BPC, KN], f32, kind="ExternalInput").ap()
    out = nc.dram_tensor("out", [BPC, M, K], f32, kind="ExternalOutput").ap()

    with tile.TileContext(nc) as tc:
        with (
            tc.tile_pool(name="tr", bufs=2) as tr_pool,
            tc.tile_pool(name="te", bufs=2) as te_pool,
            tc.tile_pool(name="single", bufs=1) as single,
            tc.tile_pool(name="outp", bufs=4) as out_pool,
            tc.tile_pool(name="ps", bufs=4, space="PSUM") as psum_pool,
        ):
            ones = single.tile([1, P], f32)
            nc.vector.memset(ones, 1.0)
            ns_all = single.tile([1, BPC, KN], f32)
            nc.sync.dma_start(out=ns_all, in_=nsq)

            for b in range(BPC):
                te = te_pool.tile([P, DT, M], f32)
                nc.sync.dma_start(
                    out=te, in_=testt[b].rearrange("(dt p) m -> p dt m", p=P)
                )
                tr = tr_pool.tile([P, DT, KN], f32)
                nc.sync.dma_start(
                    out=tr, in_=train2t[b].rearrange("(dt p) kn -> p dt kn", p=P)
                )

                for mt in range(MT):
                    psums = [
                        psum_pool.tile([P, 512], f32, name=f"psum{kc}")
                        for kc in range(NKC)
                    ]
                    for dt in range(DT):
                        lhsT = te[:, dt, mt * P:(mt + 1) * P]
                        for kc in range(NKC):
                            nc.tensor.matmul(
                                psums[kc],
                                lhsT,
                                tr[:, dt, kc * 512:(kc + 1) * 512],
                                start=(dt == 0),
                                stop=False,
                            )
                    for kc in range(NKC):
                        nc.tensor.matmul(
                            psums[kc],
                            ones,
                            ns_all[:, b, kc * 512:(kc + 1) * 512],
                            start=False,
                            stop=True,
                        )
                    otile = out_pool.tile([P, K], f32)
                    for kc in range(NKC):
                        nc.vector.reduce_max(
                            out=otile[:, kc * 32:(kc + 1) * 32],
                            in_=psums[kc].rearrange("p (k n) -> p k n", n=N),
                            axis=mybir.AxisListType.X,
                        )
                    nc.sync.dma_start(
                        out=out[b, mt * P:(mt + 1) * P, :], in_=otile
                    )
    nc.compile()
    return nc


def _get_compiled():
    global _compiled
    if _compiled is None:
        _compiled = _build()
    return _compiled


def kernel(train_inputs: np.ndarray, test_inputs: np.ndarray) -> np.ndarray:
    train = np.asarray(train_inputs, dtype=np.float32).reshape(B, KN, D)
    test = np.asarray(test_inputs, dtype=np.float32)

    train2t = np.ascontiguousarray((2.0 * train).transpose(0, 2, 1))  # [B, D, KN]
    testt = np.ascontiguousarray(test.transpose(0, 2, 1))             # [B, D, M]
    nsq = -np.einsum("bkd,bkd->bk", train, train)                     # [B, KN]
    tsq = np.einsum("bmd,bmd->bm", test, test)                        # [B, M]

    nc = _get_compiled()
    in_maps = [
        {
            "train2t": train2t[c * BPC:(c + 1) * BPC],
            "testt": testt[c * BPC:(c + 1) * BPC],
            "nsq": np.ascontiguousarray(nsq[c * BPC:(c + 1) * BPC]).reshape(1, BPC, KN),
        }
        for c in range(NCORES)
    ]
    res = run_bass_kernel_spmd(nc, in_maps, core_ids=list(range(NCORES)))
    full = np.concatenate([res.results[c]["out"] for c in range(NCORES)], axis=0)
    return (full - tsq[:, :, None]).astype(np.float32)


# revision 5
# speedup vs baseline: 3.1930x; 3.1930x over previous
"""Nearest-exemplar kNN scores on 8 Trainium2 NeuronCores.

Computes class_scores[b, m, k] = max_n( -||test[b,m] - train[b,k,n]||^2 )
data-parallel over the episode dim b (8 episodes per core).

Device kernel (per core, 8 episodes):
  psum[128m, 512kn] = sum_d testT[d, m].T @ (2*trainT[d, kn])   (4 matmuls)
                    + ones[1,128m].T @ (-train_sq[1, 512kn])    (K=1 matmul)
  out[m, k] = reduce_max over the 16 shots (innermost free axis)
The -test_sq[b, m] term is constant across (k, n), so it commutes with the
max and is applied on the host after gathering.
"""

import numpy as np

import concourse.bass as bass
import concourse.tile as tile
from concourse import bacc, mybir
from concourse.bass_utils import run_bass_kernel_spmd

B, K, N, M, D = 64, 64, 16, 512, 512
KN = K * N            # 1024 support vectors per episode
NCORES = 8
BPC = B // NCORES     # 8 episodes per core
P = 128
DT = D // P           # 4 contraction tiles
MT = M // P           # 4 query tiles
NKC = KN // 512       # 2 support chunks of 512

_compiled = None


def _build():
    nc = bacc.Bacc("TRN2", target_bir_lowering=False, debug=False, num_devices=1)
    f32 = mybir.dt.float32
    f16 = mybir.dt.float16
    train2t = nc.dram_tensor("train2t", [BPC, D, KN], f16, kind="ExternalInput").ap()
    testt = nc.dram_tensor("testt", [BPC, D, M], f16, kind="ExternalInput").ap()
    nsq = nc.dram_tensor("nsq", [1, BPC, KN], f16, kind="ExternalInput").ap()
    out = nc.dram_tensor("out", [BPC, M, K], f32, kind="ExternalOutput").ap()

    with tile.TileContext(nc) as tc:
        with (
            tc.tile_pool(name="tr", bufs=2) as tr_pool,
            tc.tile_pool(name="te", bufs=2) as te_pool,
            tc.tile_pool(name="single", bufs=1) as single,
            tc.tile_pool(name="outp", bufs=4) as out_pool,
            tc.tile_pool(name="ps", bufs=4, space="PSUM") as psum_pool,
        ):
            ones = single.tile([1, P], f16)
            nc.vector.memset(ones, 1.0)
            ns_all = single.tile([1, BPC, KN], f16)
            nc.sync.dma_start(out=ns_all, in_=nsq)

            for b in range(BPC):
                te = te_pool.tile([P, DT, M], f16)
                nc.sync.dma_start(
                    out=te, in_=testt[b].rearrange("(dt p) m -> p dt m", p=P)
                )
                tr = tr_pool.tile([P, DT, KN], f16)
                nc.sync.dma_start(
                    out=tr, in_=train2t[b].rearrange("(dt p) kn -> p dt kn", p=P)
                )

                for mt in range(MT):
                    psums = [
                        psum_pool.tile([P, 512], f32, name=f"psum{kc}")
                        for kc in range(NKC)
                    ]
                    for dt in range(DT):
                        lhsT = te[:, dt, mt * P:(mt + 1) * P]
                        for kc in range(NKC):
                            nc.tensor.matmul(
                                psums[kc],
                                lhsT,
                                tr[:, dt, kc * 512:(kc + 1) * 512],
                                start=(dt == 0),
                                stop=False,
                            )
                    for kc in range(NKC):
                        nc.tensor.matmul(
                            psums[kc],
                            ones,
                            ns_all[:, b, kc * 512:(kc + 1) * 512],
                            start=False,
                            stop=True,
                        )
                    otile = out_pool.tile([P, K], f32)
                    for kc in range(NKC):
                        nc.vector.reduce_max(
                            out=otile[:, kc * 32:(kc + 1) * 32],
                            in_=psums[kc].rearrange("p (k n) -> p k n", n=N),
                            axis=mybir.AxisListType.X,
                        )
                    nc.sync.dma_start(
                        out=out[b, mt * P:(mt + 1) * P, :], in_=otile
                    )
    nc.compile()
    return nc


def _get_compiled():
    global _compiled
    if _compiled is None:
        _compiled = _build()
    return _compiled


def _make_in_maps(train_inputs: np.ndarray, test_inputs: np.ndarray):
    train = np.asarray(train_inputs, dtype=np.float32).reshape(B, KN, D)
    test = np.asarray(test_inputs, dtype=np.float32)

    train2t = (2.0 * train).transpose(0, 2, 1).astype(np.float16)     # [B, D, KN]
    testt = test.transpose(0, 2, 1).astype(np.float16)                # [B, D, M]
    nsq = (-np.einsum("bkd,bkd->bk", train, train)).astype(np.float16)  # [B, KN]
    tsq = np.einsum("bmd,bmd->bm", test, test)                        # [B, M]

    in_maps = [
        {
            "train2t": np.ascontiguousarray(train2t[c * BPC:(c + 1) * BPC]),
            "testt": np.ascontiguousarray(testt[c * BPC:(c + 1) * BPC]),
            "nsq": np.ascontiguousarray(nsq[c * BPC:(c + 1) * BPC]).reshape(1, BPC, KN),
        }
        for c in range(NCORES)
    ]
    return in_maps, tsq


def kernel(train_inputs: np.ndarray, test_inputs: np.ndarray) -> np.ndarray:
    in_maps, tsq = _make_in_maps(train_inputs, test_inputs)
    nc = _get_compiled()
    res = run_bass_kernel_spmd(nc, in_maps, core_ids=list(range(NCORES)))
    full = np.concatenate([res.results[c]["out"] for c in range(NCORES)], axis=0)
    return (full - tsq[:, :, None]).astype(np.float32)


# revision 6
# speedup vs baseline: 3.2513x; 1.0183x over previous
"""Nearest-exemplar kNN scores on 8 Trainium2 NeuronCores.

Computes class_scores[b, m, k] = max_n( -||test[b,m] - train[b,k,n]||^2 )
data-parallel over the episode dim b (8 episodes per core).

Device kernel (per core, 8 episodes):
  psum[128m, 512kn] = sum_d testT[d, m].T @ (2*trainT[d, kn])   (4 matmuls)
                    + ones[1,128m].T @ (-train_sq[1, 512kn])    (K=1 matmul)
  out[m, k] = reduce_max over the 16 shots (innermost free axis)
The -test_sq[b, m] term is constant across (k, n), so it commutes with the
max and is applied on the host after gathering.
"""

import numpy as np

import concourse.bass as bass
import concourse.tile as tile
from concourse import bacc, mybir
from concourse.bass_utils import run_bass_kernel_spmd

B, K, N, M, D = 64, 64, 16, 512, 512
KN = K * N            # 1024 support vectors per episode
NCORES = 8
BPC = B // NCORES     # 8 episodes per core
P = 128
DT = D // P           # 4 contraction tiles
MT = M // P           # 4 query tiles
NKC = KN // 512       # 2 support chunks of 512

_compiled = None


def _build():
    nc = bacc.Bacc("TRN2", target_bir_lowering=False, debug=False, num_devices=1)
    f32 = mybir.dt.float32
    f16 = mybir.dt.float16
    train2t = nc.dram_tensor("train2t", [BPC, D, KN], f16, kind="ExternalInput").ap()
    testt = nc.dram_tensor("testt", [BPC, D, M], f16, kind="ExternalInput").ap()
    nsq = nc.dram_tensor("nsq", [1, BPC, KN], f16, kind="ExternalInput").ap()
    out = nc.dram_tensor("out", [BPC, M, K], f32, kind="ExternalOutput").ap()

    with tile.TileContext(nc) as tc:
        with (
            tc.tile_pool(name="tr", bufs=2) as tr_pool,
            tc.tile_pool(name="te", bufs=2) as te_pool,
            tc.tile_pool(name="single", bufs=1) as single,
            tc.tile_pool(name="outp", bufs=4) as out_pool,
            tc.tile_pool(name="ps", bufs=4, space="PSUM") as psum_pool,
        ):
            ones = single.tile([1, P], f16)
            nc.vector.memset(ones, 1.0)
            ns_all = single.tile([1, BPC, KN], f16)
            nc.gpsimd.dma_start(out=ns_all, in_=nsq)

            for b in range(BPC):
                # per-d-tile loads, interleaved across the two HWDGE rings so
                # the first matmul only waits for the first 384 KB
                tes, trs = [], []
                for dt in range(DT):
                    te = te_pool.tile([P, M], f16, name=f"te{dt}")
                    nc.scalar.dma_start(out=te, in_=testt[b, dt * P:(dt + 1) * P, :])
                    tes.append(te)
                    tr = tr_pool.tile([P, KN], f16, name=f"tr{dt}")
                    nc.sync.dma_start(out=tr, in_=train2t[b, dt * P:(dt + 1) * P, :])
                    trs.append(tr)

                for mt in range(MT):
                    psum = psum_pool.tile([P, KN], f32, name="psum")
                    for dt in range(DT):
                        lhsT = tes[dt][:, mt * P:(mt + 1) * P]
                        for kc in range(NKC):
                            nc.tensor.matmul(
                                psum[:, kc * 512:(kc + 1) * 512],
                                lhsT,
                                trs[dt][:, kc * 512:(kc + 1) * 512],
                                start=(dt == 0),
                                stop=False,
                            )
                    for kc in range(NKC):
                        nc.tensor.matmul(
                            psum[:, kc * 512:(kc + 1) * 512],
                            ones,
                            ns_all[:, b, kc * 512:(kc + 1) * 512],
                            start=False,
                            stop=True,
                        )
                    otile = out_pool.tile([P, K], f32)
                    nc.vector.reduce_max(
                        out=otile,
                        in_=psum.rearrange("p (k n) -> p k n", n=N),
                        axis=mybir.AxisListType.X,
                    )
                    nc.gpsimd.dma_start(
                        out=out[b, mt * P:(mt + 1) * P, :], in_=otile
                    )
    nc.compile()
    return nc


def _get_compiled():
    global _compiled
    if _compiled is None:
        _compiled = _build()
    return _compiled


def _make_in_maps(train_inputs: np.ndarray, test_inputs: np.ndarray):
    train = np.asarray(train_inputs, dtype=np.float32).reshape(B, KN, D)
    test = np.asarray(test_inputs, dtype=np.float32)

    train2t = (2.0 * train).transpose(0, 2, 1).astype(np.float16)     # [B, D, KN]
    testt = test.transpose(0, 2, 1).astype(np.float16)                # [B, D, M]
    nsq = (-np.einsum("bkd,bkd->bk", train, train)).astype(np.float16)  # [B, KN]
    tsq = np.einsum("bmd,bmd->bm", test, test)                        # [B, M]

    in_maps = [
        {
            "train2t": np.ascontiguousarray(train2t[c * BPC:(c + 1) * BPC]),
            "testt": np.ascontiguousarray(testt[c * BPC:(c + 1) * BPC]),
            "nsq": np.ascontiguousarray(nsq[c * BPC:(c + 1) * BPC]).reshape(1, BPC, KN),
        }
        for c in range(NCORES)
    ]
    return in_maps, tsq


def kernel(train_inputs: np.ndarray, test_inputs: np.ndarray) -> np.ndarray:
    in_maps, tsq = _make_in_maps(train_inputs, test_inputs)
    nc = _get_compiled()
    res = run_bass_kernel_spmd(nc, in_maps, core_ids=list(range(NCORES)))
    full = np.concatenate([res.results[c]["out"] for c in range(NCORES)], axis=0)
    return (full - tsq[:, :, None]).astype(np.float32)


# revision 9
# speedup vs baseline: 3.3599x; 1.0334x over previous
"""Nearest-exemplar kNN scores on 8 Trainium2 NeuronCores.

Computes class_scores[b, m, k] = max_n( -||test[b,m] - train[b,k,n]||^2 )
data-parallel over the episode dim b (8 episodes per core).

Device kernel (per core, 8 episodes):
  psum[128m, 512kn] = sum_d testT[d, m].T @ (2*trainT[d, kn])   (4 matmuls)
                    + ones[1,128m].T @ (-train_sq[1, 512kn])    (K=1 matmul)
  out[m, k] = reduce_max over the 16 shots (innermost free axis)
The -test_sq[b, m] term is constant across (k, n), so it commutes with the
max and is applied on the host after gathering.
"""

import numpy as np

import concourse.bass as bass
import concourse.tile as tile
from concourse import bacc, mybir
from concourse.bass_utils import run_bass_kernel_spmd

B, K, N, M, D = 64, 64, 16, 512, 512
KN = K * N            # 1024 support vectors per episode
NCORES = 8
BPC = B // NCORES     # 8 episodes per core
P = 128
DT = D // P           # 4 contraction tiles
MT = M // P           # 4 query tiles
NKC = KN // 512       # 2 support chunks of 512

_compiled = None


def _build():
    nc = bacc.Bacc("TRN2", target_bir_lowering=False, debug=False, num_devices=1)
    f32 = mybir.dt.float32
    f16 = mybir.dt.float16
    train2t = nc.dram_tensor("train2t", [BPC, D, KN], f16, kind="ExternalInput").ap()
    testt = nc.dram_tensor("testt", [BPC, D, M], f16, kind="ExternalInput").ap()
    nsq = nc.dram_tensor("nsq", [1, BPC, KN], f16, kind="ExternalInput").ap()
    out = nc.dram_tensor("out", [BPC, M, K], f32, kind="ExternalOutput").ap()

    with tile.TileContext(nc) as tc:
        with (
            tc.tile_pool(name="tr", bufs=2) as tr_pool,
            tc.tile_pool(name="te", bufs=2) as te_pool,
            tc.tile_pool(name="single", bufs=1) as single,
            tc.tile_pool(name="outp", bufs=4) as out_pool,
            tc.tile_pool(name="ps", bufs=4, space="PSUM") as psum_pool,
        ):
            ones = single.tile([1, P], f16)
            nc.vector.memset(ones, 1.0)
            ns_all = single.tile([1, BPC, KN], f16)
            nc.gpsimd.dma_start(out=ns_all, in_=nsq)

            for b in range(BPC):
                # per-d-tile loads, interleaved across the two HWDGE rings so
                # the first matmul only waits for the first 384 KB
                tes, trs = [], []
                for dt in range(DT):
                    te = te_pool.tile([P, M], f16, name=f"te{dt}")
                    nc.scalar.dma_start(out=te, in_=testt[b, dt * P:(dt + 1) * P, :])
                    tes.append(te)
                    tr = tr_pool.tile([P, KN], f16, name=f"tr{dt}")
                    nc.sync.dma_start(out=tr, in_=train2t[b, dt * P:(dt + 1) * P, :])
                    trs.append(tr)

                psums = []
                for mt in range(MT):
                    psum = psum_pool.tile([P, KN], f32, name="psum")
                    psums.append(psum)
                    for dt in range(DT):
                        lhsT = tes[dt][:, mt * P:(mt + 1) * P]
                        for kc in range(NKC):
                            nc.tensor.matmul(
                                psum[:, kc * 512:(kc + 1) * 512],
                                lhsT,
                                trs[dt][:, kc * 512:(kc + 1) * 512],
                                start=(dt == 0),
                                stop=False,
                            )
                # one LDWEIGHTS(ones) serves all 8 correction matmuls
                otile = out_pool.tile([P, MT, K], f32)
                for mt in range(MT):
                    for kc in range(NKC):
                        nc.tensor.matmul(
                            psums[mt][:, kc * 512:(kc + 1) * 512],
                            ones,
                            ns_all[:, b, kc * 512:(kc + 1) * 512],
                            start=False,
                            stop=True,
                        )
                for mt in range(MT):
                    nc.vector.reduce_max(
                        out=otile[:, mt, :],
                        in_=psums[mt].rearrange("p (k n) -> p k n", n=N),
                        axis=mybir.AxisListType.X,
                    )
                nc.gpsimd.dma_start(
                    out=out[b].rearrange("(mt p) k -> p mt k", p=P), in_=otile
                )
    nc.compile()
    return nc


def _get_compiled():
    global _compiled
    if _compiled is None:
        _compiled = _build()
    return _compiled


def _make_in_maps(train_inputs: np.ndarray, test_inputs: np.ndarray):
    train = np.asarray(train_inputs, dtype=np.float32).reshape(B, KN, D)
    test = np.asarray(test_inputs, dtype=np.float32)

    train2t = (2.0 * train).transpose(0, 2, 1).astype(np.float16)     # [B, D, KN]
    testt = test.transpose(0, 2, 1).astype(np.float16)                # [B, D, M]
    nsq = (-np.einsum("bkd,bkd->bk", train, train)).astype(np.float16)  # [B, KN]
    tsq = np.einsum("bmd,bmd->bm", test, test)                        # [B, M]

    in_maps = [
        {
            "train2t": np.ascontiguousarray(train2t[c * BPC:(c + 1) * BPC]),
            "testt": np.ascontiguousarray(testt[c * BPC:(c + 1) * BPC]),
            "nsq": np.ascontiguousarray(nsq[c * BPC:(c + 1) * BPC]).reshape(1, BPC, KN),
        }
        for c in range(NCORES)
    ]
    return in_maps, tsq


def kernel(train_inputs: np.ndarray, test_inputs: np.ndarray) -> np.ndarray:
    in_maps, tsq = _make_in_maps(train_inputs, test_inputs)
    nc = _get_compiled()
    res = run_bass_kernel_spmd(nc, in_maps, core_ids=list(range(NCORES)))
    full = np.concatenate([res.results[c]["out"] for c in range(NCORES)], axis=0)
    return (full - tsq[:, :, None]).astype(np.float32)
